# revision 41
# baseline (speedup 1.0000x reference)
"""Trainium2 Bass kernel: CenterHeadIoU 1x1-conv stack (bf16 v3, PE-tiled).

Computes, for x = ct_feat [B=32, C=128, N=8192]:
  y = relu(bn(sh_w @ x))                       [B, 64, N]
  z_h = relu(bn_h(head_w1[h] @ y)), h=0..5     [B, 64, N] each
  out = concat_h(head_final_w[h] @ z_h + b_h)  [B, 12, N]

Sharding: data-parallel over batch, 4 batches per core on 8 cores;
weights tiny and replicated. BN folded on the host; x and weights cast
to bf16 (rel err ~7e-4 vs the 2e-2 gate). On this part the PE streams
512 columns in ~427ns regardless of dtype (1.2 GHz), so the design
minimizes full-width matmul slots via PE array tiling (concurrent
tiles at distinct tile_positions):

Per group of 4 tiles (F=512 cols each):
  mm1 (128,32 mode): y for tile-pairs of the NEXT group — 4 col-tiled
       M=32 matmuls share one slot, packing 2 tiles' y into one
       [128,F] psum bank; one ACT op evacuates both (relu+bias).
  mm2 (64,128 mode): pair p of tile t runs on row half t%2 (y(t) lives
       in sbuf partitions 0-63 or 64-127), so two tiles' pair-matmuls
       run concurrently (12 matmuls in ~6 slots), sharing [128,2F]
       psum tiles (row halves write different banks). K=64 (no bias
       row): the first-layer bias is applied by the merged [128,2F]
       PSUM-evac ops (per-partition bias AP, alternating ACT/DVE).
  mm3 (128,32 mode): 3 waves of 4 col-tiled accumulating matmuls
       (M=12 at positions 0/32/64/96) pack 4 tiles' outputs into ONE
       psum bank, pre-filled with b3 by a rank-1 "bias matmul" (sets
       has_written everywhere, so the waves use start=False and the
       final evac is a plain copy + 4 out-DMAs).
Two-behind software pipeline: mm3 consumes z from TWO groups back so
epi is DVE's first op each cycle with its input ready at cycle start,
and the bias matmul is emitted a cycle late so 1-bank py/po pools plus
a stationary 3x[128,2F] pz ring use exactly the 8 PSUM banks. Queue
placement: out-DMAs + first x loads on the sync queue (HWDGE), x
prefetch + cold consts on the gpsimd queue (SWDGE latency hidden by
the 2-cycle prefetch window).
A post-pass moves multi-wait sync conditions onto single-wait NoOp
carriers (this walrus build caps sync waits per instruction).
"""

import os
import sys
import numpy as np

B, C_IN, N, HC = 32, 128, 8192, 64
NCORES = 8
BC = B // NCORES            # batches per core
F = 512                     # free-dim tile = one fp32 PSUM bank
NT = N // F                 # tiles per batch
T = BC * NT                 # tiles per core
G = 4                       # tiles per out-group (4 col positions)
NG = T // G
EPS = 1e-5
HEAD_OUT = [3, 2, 1, 3, 2, 1]        # hm, reg, height, dim, rot, iou
PAIR_OFF = [0, 5, 9]                 # channel offset of pair p in the 12-ch output

_CACHE = {}
LAST_RESULTS = None
LAST_EXEC_NS = None


def _build_program():
    import concourse.bass as bass
    import concourse.mybir as mybir
    import concourse.tile as tile

    f32 = mybir.dt.float32
    bf16 = mybir.dt.bfloat16
    AF = mybir.ActivationFunctionType
    OP = mybir.AluOpType

    nc = bass.Bass("TRN2", target_bir_lowering=False, debug=False,
                   num_devices=NCORES)

    x = nc.dram_tensor("x", [BC, C_IN, N], bf16, kind="ExternalInput").ap()
    w1 = nc.dram_tensor("w1", [C_IN, 128], bf16, kind="ExternalInput").ap()
    b1 = nc.dram_tensor("b1", [128, 1], f32, kind="ExternalInput").ap()
    w2 = nc.dram_tensor("w2", [C_IN, 384], bf16, kind="ExternalInput").ap()
    b2 = nc.dram_tensor("b2", [128, 3], f32, kind="ExternalInput").ap()
    w3 = nc.dram_tensor("w3", [C_IN, 36], bf16, kind="ExternalInput").ap()
    b3r = nc.dram_tensor("b3r", [HC, 128], bf16, kind="ExternalInput").ap()
    onesk = nc.dram_tensor("onesk", [HC, F], bf16, kind="ExternalInput").ap()
    out = nc.dram_tensor("out", [BC, 12, N], f32, kind="ExternalOutput").ap()

    with tile.TileContext(nc) as tc:
        with (
            tc.tile_pool(name="consts", bufs=1) as cpool,
            tc.tile_pool(name="xin", bufs=4) as xpool,
            tc.tile_pool(name="ysb", bufs=2) as ypool,
            tc.tile_pool(name="zsb", bufs=6) as zpool,
            tc.tile_pool(name="osb", bufs=2) as opool,
            tc.tile_pool(name="ps", bufs=3, space="PSUM") as ppool,
            tc.tile_pool(name="py", bufs=1, space="PSUM") as pypool,
            tc.tile_pool(name="po", bufs=1, space="PSUM") as popool,
        ):
            # x(0)/x(1) go first on the sync queue (longest transfers on the
            # critical path to the first matmul); w1/b1 follow; the other
            # consts ride the otherwise-idle gpsimd queue.
            xts = {}

            def load_group(g, split=False):
                if g >= NG:
                    return
                b, jg = divmod(g, NT // G)
                xt = xpool.tile([C_IN, G * F], bf16, name="xt", tag="xt")
                if split:
                    # halve the first transfer so the first mm1 pair can
                    # start as soon as its two tiles land
                    nc.sync.dma_start(
                        out=xt[:, 0:2 * F],
                        in_=x[b, :, jg * G * F:jg * G * F + 2 * F])
                    nc.sync.dma_start(
                        out=xt[:, 2 * F:G * F],
                        in_=x[b, :, jg * G * F + 2 * F:(jg + 1) * G * F])
                else:
                    # steady-state x prefetch rides the otherwise-idle
                    # gpsimd queue (its SWDGE latency hides in the 2-cycle
                    # prefetch window), keeping the sync queue free for the
                    # latency-sensitive out-DMAs
                    nc.gpsimd.dma_start(
                        out=xt[:], in_=x[b, :, jg * G * F:(jg + 1) * G * F])
                xts[g] = xt

            # first mm1 needs x tiles 0-1 and w1: start that transfer first,
            # then w1/b1, then the rest of group 0 and group 1. (Do NOT put
            # any DMA on the scalar queue — the kernel is ACT-bound and it
            # measurably disrupts the ScalarE pipeline.)
            xt0 = xpool.tile([C_IN, G * F], bf16, name="xt", tag="xt")
            nc.sync.dma_start(out=xt0[:], in_=x[0, :, 0:G * F])
            xts[0] = xt0
            w1_t = cpool.tile([C_IN, 128], bf16, name="w1_t")
            nc.sync.dma_start(out=w1_t[:], in_=w1[:])
            b1_t = cpool.tile([128, 1], f32, name="b1_t")
            nc.sync.dma_start(out=b1_t[:], in_=b1[:])
            load_group(1, split=True)
            w2_t = cpool.tile([C_IN, 384], bf16, name="w2_t")
            nc.gpsimd.dma_start(out=w2_t[:], in_=w2[:])
            b2_t = cpool.tile([128, 3], f32, name="b2_t")
            nc.gpsimd.dma_start(out=b2_t[:], in_=b2[:])
            w3_t = cpool.tile([C_IN, 36], bf16, name="w3_t")
            nc.gpsimd.dma_start(out=w3_t[:], in_=w3[:])
            b3r_t = cpool.tile([HC, 128], bf16, name="b3r_t")
            nc.gpsimd.dma_start(out=b3r_t[:], in_=b3r[:])
            onesk_t = cpool.tile([HC, F], bf16, name="onesk_t")
            nc.gpsimd.dma_start(out=onesk_t[:], in_=onesk[:])

            yts = {}
            zsps = {}
            pos = {}

            def psum():
                return ppool.tile([C_IN, 2 * F], f32, name="ps", tag="ps")

            pend_act1 = {}

            def act1(g, pairidx, py):
                yt = ypool.tile([C_IN, F], bf16, name=f"y{pairidx}",
                                tag=f"y{pairidx}")
                nc.scalar.activation(yt[:, :], py[:, :], AF.Relu,
                                     bias=b1_t[:, 0:1], scale=1.0)
                yts[(g, pairidx)] = yt

            def flush_act1(g):
                # deferred pair-B evac: emitted at the start of the next
                # cycle so it is ACT's first op with its input (written by
                # the previous cycle's mm1b) already available
                if g in pend_act1:
                    act1(g, 1, pend_act1.pop(g))

            def mm1_pair(g, pairidx, defer=False, pool=None):
                # (128,32) slot: y for tiles (2*pairidx, 2*pairidx+1) of
                # group g via 4-way col tiling into ONE [128,F] psum bank;
                # one ACT op evacuates both tiles' y (relu+bias).
                if g >= NG:
                    return
                xt = xts[g]
                # pool override: the warmup pair B borrows the po bank
                # (idle until cycle 1) so it needn't wait for act1a; the
                # tag must match bias_mm's so the single buffer is shared.
                if pool is None:
                    py = pypool.tile([C_IN, F], f32, name="py", tag="py")
                else:
                    py = pool.tile([C_IN, F], f32, name="py", tag="po")
                for q in range(4):          # col tiles q0/q32/q64/q96
                    e = q // 2              # tile within the pair
                    nc.tensor.matmul(
                        py[32 * q:32 * (q + 1), :],
                        w1_t[:, 32 * q:32 * (q + 1)],
                        xt[:, (2 * pairidx + e) * F:(2 * pairidx + e + 1) * F],
                        start=True, stop=True,
                        tile_position=(0, 32 * q))
                if defer:
                    pend_act1[g] = py
                else:
                    act1(g, pairidx, py)

            def mm2_pair(g, pairidx):
                # (64,128) slots: pair-matmuls for tiles (4g+2*pairidx,
                # 4g+2*pairidx+1); tile parity picks the row half AND the
                # psum col half, so the two tiles overlap on the array and
                # write different banks.
                yt = yts.pop((g, pairidx))
                zsp = zpool.tile([C_IN, 6 * F], bf16, name="zsp", tag="zsp")
                pzs = []
                for p in range(3):
                    pz = psum()
                    for e in range(2):
                        h = HC * e
                        nc.tensor.matmul(
                            pz[:, e * F:(e + 1) * F],
                            w2_t[h:h + HC, 128 * p:128 * (p + 1)],
                            yt[h:h + HC, :],
                            start=True, stop=True)
                    pzs.append(pz)
                # relu + first-layer bias (per-partition AP, same for both
                # tiles of the pair); one [128,2F] op per pair p,
                # alternating ACT/DVE
                for p in range(3):
                    if (p + pairidx) % 2 == 0:
                        nc.scalar.activation(
                            zsp[:, 2 * p * F:(2 * p + 2) * F],
                            pzs[p][:, :], AF.Relu,
                            bias=b2_t[:, p:p + 1], scale=1.0)
                    else:
                        nc.vector.tensor_scalar(
                            zsp[:, 2 * p * F:(2 * p + 2) * F],
                            pzs[p][:, :], b2_t[:, p:p + 1], 0.0,
                            OP.add, OP.max)
                zsps[2 * g + pairidx] = zsp

            def bias_mm(g, pool=None, tag="po"):
                # rank-1 bias matmul: fills the out bank for group g's mm3
                # with b3 and sets has_written on every element. The LAST
                # group borrows the py bank (idle once the final act1 ran)
                # so its bias->waves chain overlaps epi(NG-2) in the drain.
                po = (pool or popool).tile([C_IN, F], f32, name="po",
                                           tag=tag)
                nc.tensor.matmul(po[:, :], b3r_t[:, :], onesk_t[:, :],
                                 start=True, stop=True, skip_group_check=True)
                pos[g] = po

            def mm3_group(g):
                # (128,32) stretch: 3 waves of 4 col-tiled accumulating
                # matmuls; z is one group old so the PE never waits here.
                po = pos.pop(g)
                for p in range(3):
                    for i in range(G):
                        zsp = zsps[2 * g + i // 2]
                        e = i % 2
                        nc.tensor.matmul(
                            po[32 * i:32 * i + 12, :],
                            w3_t[:, 12 * p:12 * (p + 1)],
                            zsp[:, (2 * p + e) * F:(2 * p + e + 1) * F],
                            start=False, stop=(p == 2 and i == G - 1),
                            skip_group_check=True,
                            tile_position=(0, 32 * i))
                return po

            def epi_group(g, po):
                b, jg = divmod(g, NT // G)
                ot = opool.tile([128, F], f32, name="ot", tag="ot")
                nc.vector.tensor_scalar_add(ot[:, :], po[:, :], 0.0)
                # the final groups' DMAs are the kernel's tail: spread them
                # across four queues so their dispatches run in parallel
                # (earlier groups stay on sync where dispatch cost hides)
                if g >= NG - 2:
                    queues = [nc.sync, nc.gpsimd, nc.scalar, nc.sync]
                else:
                    queues = [nc.sync] * G
                for i in range(G):
                    j = jg * G + i
                    queues[i].dma_start(out=out[b, :, j * F:(j + 1) * F],
                                        in_=ot[32 * i:32 * i + 12, :])
                zsps.pop(2 * g, None)
                zsps.pop(2 * g + 1, None)

            # Pipeline: waves+bias for group g-1 run at the END of cycle g
            # (z(g-1) was fully evacuated during cycle g-1), so epi(g-2) is
            # DVE's first op of cycle g with its input already written, and
            # the deferred act1b(g) is ACT's first op likewise. Engine
            # queues per cycle:
            # PE  [pzA(g), mm1a(g+1), pzB(g), bias(g-1), waves(g-1),
            #      mm1b(g+1)]
            # ACT [act1b(g), zA0, zA2, act1a(g+1), zB1]
            # DVE [epi(g-2), zA1, zB0, zB2]
            wavepo = {}
            mm1_pair(0, 0)
            mm1_pair(0, 1, pool=popool)
            for g in range(NG):
                flush_act1(g)
                if g >= 2:
                    epi_group(g - 2, wavepo.pop(g - 2))
                load_group(g + 2)
                mm2_pair(g, 0)
                mm1_pair(g + 1, 0)
                mm2_pair(g, 1)
                if g >= 1:
                    bias_mm(g - 1)
                    wavepo[g - 1] = mm3_group(g - 1)
                mm1_pair(g + 1, 1, defer=True)
            bias_mm(NG - 1, pool=pypool, tag="py")
            epi_group(NG - 2, wavepo.pop(NG - 2))
            wavepo[NG - 1] = mm3_group(NG - 1)
            epi_group(NG - 1, wavepo.pop(NG - 1))
    _split_waits(nc)
    return nc


def _split_waits(nc, cap=1):
    """This container's walrus build rejects instructions carrying more than
    a small number of sync waits (matmuls: just one). Move excess waits onto
    single-wait NoOp carriers inserted before the instruction on the same
    engine — semantically identical (conjunction of waits, in-order
    sequencers)."""
    import concourse.mybir as mybir

    k = 0
    for func in nc.m.functions:
        for bb in func.blocks:
            insts = bb.instructions
            out_insts = []
            changed = False
            for inst in insts:
                si = inst.sync_info
                waits = list(si.on_wait) if si and si.on_wait else []
                if len(waits) > cap:
                    for w in waits[:-cap]:
                        d = mybir.InstNoOp(name=f"I-sw{k}", ins=[], outs=[])
                        k += 1
                        d.engine = inst.engine
                        d.sync_info = mybir.SyncInfo(on_wait=[w], on_update=[])
                        nc.register_instruction(d)
                        out_insts.append(d)
                    inst.sync_info = mybir.SyncInfo(
                        on_wait=waits[-cap:],
                        on_update=list(si.on_update) if si.on_update else [])
                    changed = True
                out_insts.append(inst)
            if changed:
                bb.instructions = out_insts


def _get_program():
    if "nc" not in _CACHE:
        _CACHE["nc"] = _build_program()
    return _CACHE["nc"]


def _prep_weights(d):
    """Fold BN into conv weights/biases; pack stationary matrices (bf16)."""
    import ml_dtypes
    f8 = np.float64
    bf = ml_dtypes.bfloat16

    def g(name):
        return np.asarray(d[name], dtype=f8)

    # shared conv + BN
    s1 = g("sh_g") / np.sqrt(g("sh_var") + EPS)                     # [64]
    W1e = g("sh_w") * s1[:, None]                                   # [64,128]
    b1e = g("sh_b") * s1 + g("sh_beta") - g("sh_mean") * s1         # [64]
    w1 = np.zeros((C_IN, 128), f8)
    w1[:, 0:HC] = W1e.T                 # col-pair slot 0 -> psum rows 0..63
    w1[:, HC:128] = W1e.T               # col-pair slot 1 -> psum rows 64..127
    b1 = np.concatenate([b1e, b1e])[:, None]                        # [128,1]

    # head first layers + BN: K=64 row-tiled stationaries; pair p's block
    # has head 2p in output cols 0..63 and head 2p+1 in cols 64..127; the
    # weights are duplicated into both sbuf row halves (T0 and T8 copies).
    s2 = g("head_g1") / np.sqrt(g("head_var1") + EPS)               # [6,64]
    W2e = g("head_w1") * s2[:, :, None]                             # [6,64,64]
    b2e = g("head_b1") * s2 + g("head_beta1") - g("head_mean1") * s2  # [6,64]
    w2 = np.zeros((C_IN, 384), f8)
    b2 = np.zeros((128, 3), f8)
    for p in range(3):
        blk = np.zeros((HC, 128), f8)
        blk[:, 0:HC] = W2e[2 * p].T
        blk[:, HC:128] = W2e[2 * p + 1].T
        w2[0:HC, 128 * p:128 * (p + 1)] = blk
        w2[HC:128, 128 * p:128 * (p + 1)] = blk
        b2[0:HC, p] = b2e[2 * p]
        b2[HC:128, p] = b2e[2 * p + 1]

    # final convs: three accumulating M=12 blocks (pair p: head 2p from z
    # rows 0..63, head 2p+1 from rows 64..127, into its channel offsets)
    names = ["hm", "reg", "height", "dim", "rot", "iou"]
    Wf = [g(n + "_w") for n in names]
    bfin = [g(n + "_b") for n in names]
    w3 = np.zeros((C_IN, 36), f8)
    b3full = np.zeros((12,), f8)
    for p in range(3):
        ha, hb = 2 * p, 2 * p + 1
        ca, cb = HEAD_OUT[ha], HEAD_OUT[hb]
        off = PAIR_OFF[p]
        w3[0:64, 12 * p + off:12 * p + off + ca] = Wf[ha].T
        w3[64:128, 12 * p + off + ca:12 * p + off + ca + cb] = Wf[hb].T
        b3full[off:off + ca] = bfin[ha]
        b3full[off + ca:off + ca + cb] = bfin[hb]
    # rank-1 bias matmul operands: row 0 of b3r x row 0 of onesk broadcasts
    # b3 (in the 4x32 col-group layout) across the whole out bank
    b3r = np.zeros((HC, 128), f8)
    for i in range(G):
        b3r[0, 32 * i:32 * i + 12] = b3full
    onesk = np.zeros((HC, F), f8)
    onesk[0, :] = 1.0

    c = np.float32
    return {"w1": w1.astype(bf), "b1": b1.astype(c), "w2": w2.astype(bf),
            "b2": b2.astype(c), "w3": w3.astype(bf), "b3r": b3r.astype(bf),
            "onesk": onesk.astype(bf)}


def _ensure_ntff_hook():
    """Install the antenv.axon_hooks NTFF-profile shim if the container's
    antenv package lacks it (profiling only; never used in grading runs)."""
    try:
        from antenv.axon_hooks import get_axon_ntff_profile_hook  # noqa: F401
        return True
    except ImportError:
        pass
    import contextlib
    import ctypes
    import sys as _sys
    import types

    so_path = "/opt/axon/libaxon_pjrt.so"
    if not os.path.exists(so_path):
        return False
    lib = ctypes.CDLL(so_path)
    if not hasattr(lib, "axon_start_nrt_profile"):
        return False
    lib.axon_start_nrt_profile.argtypes = [ctypes.POINTER(ctypes.c_int64),
                                           ctypes.c_size_t]
    lib.axon_start_nrt_profile.restype = ctypes.c_int64
    lib.axon_stop_nrt_profile.argtypes = [ctypes.c_char_p]
    lib.axon_stop_nrt_profile.restype = ctypes.c_int64

    @contextlib.contextmanager
    def _hook(output_dir, device_ids):
        import jax
        jax.devices()
        if device_ids:
            ids = (ctypes.c_int64 * len(device_ids))(*device_ids)
            rc = lib.axon_start_nrt_profile(ids, len(device_ids))
        else:
            rc = lib.axon_start_nrt_profile(None, 0)
        if rc != 0:
            raise RuntimeError(f"axon_start_nrt_profile rc={rc}")
        try:
            yield
        finally:
            n = lib.axon_stop_nrt_profile(str(output_dir).encode())
            print(f"profile: {n} file(s) written to {output_dir}",
                  file=sys.stderr)

    import antenv
    mod = types.ModuleType("antenv.axon_hooks")
    mod.get_axon_ntff_profile_hook = lambda: _hook
    mod.set_axon_ntff_profile_hook = lambda h: None
    _sys.modules["antenv.axon_hooks"] = mod
    antenv.axon_hooks = mod
    return True


def kernel(**inputs):
    global LAST_RESULTS, LAST_EXEC_NS
    import ml_dtypes
    from concourse.bass_utils import run_bass_kernel_spmd

    inputs = {k: np.asarray(v) for k, v in inputs.items()}
    weights = _prep_weights(inputs)

    ct = np.asarray(inputs["ct_feat"], dtype=np.float32)
    xs = ct.astype(ml_dtypes.bfloat16).reshape(NCORES, BC, C_IN, N)

    in_maps = [dict(weights, x=np.ascontiguousarray(xs[i]))
               for i in range(NCORES)]

    nc = _get_program()
    trace = bool(int(os.environ.get("CK_PROFILE", "0")))
    if trace:
        trace = _ensure_ntff_hook()
    res = run_bass_kernel_spmd(nc, in_maps, list(range(NCORES)), trace=trace)
    LAST_RESULTS = res
    LAST_EXEC_NS = res.exec_time_ns

    out = np.concatenate([np.asarray(res.results[i]["out"])
                          for i in range(NCORES)], axis=0)
    return out.astype(np.float32)


# revision 43
# speedup vs baseline: 1.0144x; 1.0144x over previous
"""Trainium2 Bass kernel: CenterHeadIoU 1x1-conv stack (bf16 v3, PE-tiled).

Computes, for x = ct_feat [B=32, C=128, N=8192]:
  y = relu(bn(sh_w @ x))                       [B, 64, N]
  z_h = relu(bn_h(head_w1[h] @ y)), h=0..5     [B, 64, N] each
  out = concat_h(head_final_w[h] @ z_h + b_h)  [B, 12, N]

Sharding: data-parallel over batch, 4 batches per core on 8 cores;
weights tiny and replicated. BN folded on the host; x and weights cast
to bf16 (rel err ~7e-4 vs the 2e-2 gate). On this part the PE streams
512 columns in ~427ns regardless of dtype (1.2 GHz), so the design
minimizes full-width matmul slots via PE array tiling (concurrent
tiles at distinct tile_positions):

Per group of 4 tiles (F=512 cols each):
  mm1 (128,32 mode): y for tile-pairs of the NEXT group — 4 col-tiled
       M=32 matmuls share one slot, packing 2 tiles' y into one
       [128,F] psum bank; one ACT op evacuates both (relu+bias).
  mm2 (64,128 mode): pair p of tile t runs on row half t%2 (y(t) lives
       in sbuf partitions 0-63 or 64-127), so two tiles' pair-matmuls
       run concurrently (12 matmuls in ~6 slots), sharing [128,2F]
       psum tiles (row halves write different banks). K=64 (no bias
       row): the first-layer bias is applied by the merged [128,2F]
       PSUM-evac ops (per-partition bias AP, alternating ACT/DVE).
  mm3 (128,32 mode): 3 waves of 4 col-tiled accumulating matmuls
       (M=12 at positions 0/32/64/96) pack 4 tiles' outputs into ONE
       psum bank, pre-filled with b3 by a rank-1 "bias matmul" (sets
       has_written everywhere, so the waves use start=False and the
       final evac is a plain copy + 4 out-DMAs).
Two-behind software pipeline: mm3 consumes z from TWO groups back so
epi is DVE's first op each cycle with its input ready at cycle start,
and the bias matmul is emitted a cycle late so 1-bank py/po pools plus
a stationary 3x[128,2F] pz ring use exactly the 8 PSUM banks. Queue
placement: out-DMAs + first x loads on the sync queue (HWDGE), x
prefetch + cold consts on the gpsimd queue (SWDGE latency hidden by
the 2-cycle prefetch window).
A post-pass moves multi-wait sync conditions onto single-wait NoOp
carriers (this walrus build caps sync waits per instruction).
"""

import os
import sys
import numpy as np

B, C_IN, N, HC = 32, 128, 8192, 64
NCORES = 8
BC = B // NCORES            # batches per core
F = 512                     # free-dim tile = one fp32 PSUM bank
NT = N // F                 # tiles per batch
T = BC * NT                 # tiles per core
G = 4                       # tiles per out-group (4 col positions)
NG = T // G
EPS = 1e-5
HEAD_OUT = [3, 2, 1, 3, 2, 1]        # hm, reg, height, dim, rot, iou
PAIR_OFF = [0, 5, 9]                 # channel offset of pair p in the 12-ch output

_CACHE = {}
LAST_RESULTS = None
LAST_EXEC_NS = None


def _build_program():
    import concourse.bass as bass
    import concourse.mybir as mybir
    import concourse.tile as tile

    f32 = mybir.dt.float32
    bf16 = mybir.dt.bfloat16
    AF = mybir.ActivationFunctionType
    OP = mybir.AluOpType

    nc = bass.Bass("TRN2", target_bir_lowering=False, debug=False,
                   num_devices=NCORES)

    x = nc.dram_tensor("x", [BC, C_IN, N], bf16, kind="ExternalInput").ap()
    w1 = nc.dram_tensor("w1", [C_IN, 128], bf16, kind="ExternalInput").ap()
    b1 = nc.dram_tensor("b1", [128, 1], f32, kind="ExternalInput").ap()
    w2 = nc.dram_tensor("w2", [C_IN, 384], bf16, kind="ExternalInput").ap()
    b2 = nc.dram_tensor("b2", [128, 3], f32, kind="ExternalInput").ap()
    w3 = nc.dram_tensor("w3", [C_IN, 36], bf16, kind="ExternalInput").ap()
    b3r = nc.dram_tensor("b3r", [HC, 128], bf16, kind="ExternalInput").ap()
    onesk = nc.dram_tensor("onesk", [HC, F], bf16, kind="ExternalInput").ap()
    out = nc.dram_tensor("out", [BC, 12, N], f32, kind="ExternalOutput").ap()

    with tile.TileContext(nc) as tc:
        with (
            tc.tile_pool(name="consts", bufs=1) as cpool,
            tc.tile_pool(name="xin", bufs=4) as xpool,
            tc.tile_pool(name="ysb", bufs=2) as ypool,
            tc.tile_pool(name="zsb", bufs=6) as zpool,
            tc.tile_pool(name="osb", bufs=2) as opool,
            tc.tile_pool(name="ps", bufs=3, space="PSUM") as ppool,
            tc.tile_pool(name="py", bufs=1, space="PSUM") as pypool,
            tc.tile_pool(name="po", bufs=1, space="PSUM") as popool,
        ):
            # x(0)/x(1) go first on the sync queue (longest transfers on the
            # critical path to the first matmul); w1/b1 follow; the other
            # consts ride the otherwise-idle gpsimd queue.
            xts = {}

            def load_group(g, split=False):
                if g >= NG:
                    return
                b, jg = divmod(g, NT // G)
                xt = xpool.tile([C_IN, G * F], bf16, name="xt", tag="xt")
                if split:
                    # halve the first transfer so the first mm1 pair can
                    # start as soon as its two tiles land
                    nc.sync.dma_start(
                        out=xt[:, 0:2 * F],
                        in_=x[b, :, jg * G * F:jg * G * F + 2 * F])
                    nc.sync.dma_start(
                        out=xt[:, 2 * F:G * F],
                        in_=x[b, :, jg * G * F + 2 * F:(jg + 1) * G * F])
                else:
                    # steady-state x prefetch rides the otherwise-idle
                    # gpsimd queue (its SWDGE latency hides in the 2-cycle
                    # prefetch window), keeping the sync queue free for the
                    # latency-sensitive out-DMAs
                    nc.gpsimd.dma_start(
                        out=xt[:], in_=x[b, :, jg * G * F:(jg + 1) * G * F])
                xts[g] = xt

            # first mm1 needs x tiles 0-1 and w1: start that transfer first,
            # then w1/b1, then the rest of group 0 and group 1. (Do NOT put
            # any DMA on the scalar queue — the kernel is ACT-bound and it
            # measurably disrupts the ScalarE pipeline.)
            xt0 = xpool.tile([C_IN, G * F], bf16, name="xt", tag="xt")
            nc.sync.dma_start(out=xt0[:, 0:2 * F], in_=x[0, :, 0:2 * F])
            xts[0] = xt0
            w1_t = cpool.tile([C_IN, 128], bf16, name="w1_t")
            nc.sync.dma_start(out=w1_t[:], in_=w1[:])
            b1_t = cpool.tile([128, 1], f32, name="b1_t")
            nc.sync.dma_start(out=b1_t[:], in_=b1[:])
            nc.sync.dma_start(out=xt0[:, 2 * F:G * F],
                              in_=x[0, :, 2 * F:G * F])
            load_group(1, split=True)
            w2_t = cpool.tile([C_IN, 384], bf16, name="w2_t")
            nc.gpsimd.dma_start(out=w2_t[:], in_=w2[:])
            b2_t = cpool.tile([128, 3], f32, name="b2_t")
            nc.gpsimd.dma_start(out=b2_t[:], in_=b2[:])
            w3_t = cpool.tile([C_IN, 36], bf16, name="w3_t")
            nc.gpsimd.dma_start(out=w3_t[:], in_=w3[:])
            b3r_t = cpool.tile([HC, 128], bf16, name="b3r_t")
            nc.gpsimd.dma_start(out=b3r_t[:], in_=b3r[:])
            onesk_t = cpool.tile([HC, F], bf16, name="onesk_t")
            nc.gpsimd.dma_start(out=onesk_t[:], in_=onesk[:])

            yts = {}
            zsps = {}
            pos = {}

            def psum():
                return ppool.tile([C_IN, 2 * F], f32, name="ps", tag="ps")

            pend_act1 = {}

            def act1(g, pairidx, py):
                yt = ypool.tile([C_IN, F], bf16, name=f"y{pairidx}",
                                tag=f"y{pairidx}")
                nc.scalar.activation(yt[:, :], py[:, :], AF.Relu,
                                     bias=b1_t[:, 0:1], scale=1.0)
                yts[(g, pairidx)] = yt

            def flush_act1(g):
                # deferred pair-B evac: emitted at the start of the next
                # cycle so it is ACT's first op with its input (written by
                # the previous cycle's mm1b) already available
                if g in pend_act1:
                    act1(g, 1, pend_act1.pop(g))

            def mm1_pair(g, pairidx, defer=False, pool=None):
                # (128,32) slot: y for tiles (2*pairidx, 2*pairidx+1) of
                # group g via 4-way col tiling into ONE [128,F] psum bank;
                # one ACT op evacuates both tiles' y (relu+bias).
                if g >= NG:
                    return
                xt = xts[g]
                # pool override: the warmup pair B borrows the po bank
                # (idle until cycle 1) so it needn't wait for act1a; the
                # tag must match bias_mm's so the single buffer is shared.
                if pool is None:
                    py = pypool.tile([C_IN, F], f32, name="py", tag="py")
                else:
                    py = pool.tile([C_IN, F], f32, name="py", tag="po")
                for q in range(4):          # col tiles q0/q32/q64/q96
                    e = q // 2              # tile within the pair
                    nc.tensor.matmul(
                        py[32 * q:32 * (q + 1), :],
                        w1_t[:, 32 * q:32 * (q + 1)],
                        xt[:, (2 * pairidx + e) * F:(2 * pairidx + e + 1) * F],
                        start=True, stop=True,
                        tile_position=(0, 32 * q))
                if defer:
                    pend_act1[g] = py
                else:
                    act1(g, pairidx, py)

            def mm2_pair(g, pairidx):
                # (64,128) slots: pair-matmuls for tiles (4g+2*pairidx,
                # 4g+2*pairidx+1); tile parity picks the row half AND the
                # psum col half, so the two tiles overlap on the array and
                # write different banks.
                yt = yts.pop((g, pairidx))
                zsp = zpool.tile([C_IN, 6 * F], bf16, name="zsp", tag="zsp")
                pzs = []
                for p in range(3):
                    pz = psum()
                    for e in range(2):
                        h = HC * e
                        nc.tensor.matmul(
                            pz[:, e * F:(e + 1) * F],
                            w2_t[h:h + HC, 128 * p:128 * (p + 1)],
                            yt[h:h + HC, :],
                            start=True, stop=True)
                    pzs.append(pz)
                # relu + first-layer bias (per-partition AP, same for both
                # tiles of the pair); one [128,2F] op per pair p,
                # alternating ACT/DVE
                for p in range(3):
                    if (p + pairidx) % 2 == 0:
                        nc.scalar.activation(
                            zsp[:, 2 * p * F:(2 * p + 2) * F],
                            pzs[p][:, :], AF.Relu,
                            bias=b2_t[:, p:p + 1], scale=1.0)
                    else:
                        nc.vector.tensor_scalar(
                            zsp[:, 2 * p * F:(2 * p + 2) * F],
                            pzs[p][:, :], b2_t[:, p:p + 1], 0.0,
                            OP.add, OP.max)
                zsps[2 * g + pairidx] = zsp

            def bias_mm(g, pool=None, tag="po"):
                # rank-1 bias matmul: fills the out bank for group g's mm3
                # with b3 and sets has_written on every element. The LAST
                # group borrows the py bank (idle once the final act1 ran)
                # so its bias->waves chain overlaps epi(NG-2) in the drain.
                po = (pool or popool).tile([C_IN, F], f32, name="po",
                                           tag=tag)
                nc.tensor.matmul(po[:, :], b3r_t[:, :], onesk_t[:, :],
                                 start=True, stop=True, skip_group_check=True)
                pos[g] = po

            def mm3_group(g):
                # (128,32) stretch: 3 waves of 4 col-tiled accumulating
                # matmuls; z is one group old so the PE never waits here.
                po = pos.pop(g)
                for p in range(3):
                    for i in range(G):
                        zsp = zsps[2 * g + i // 2]
                        e = i % 2
                        nc.tensor.matmul(
                            po[32 * i:32 * i + 12, :],
                            w3_t[:, 12 * p:12 * (p + 1)],
                            zsp[:, (2 * p + e) * F:(2 * p + e + 1) * F],
                            start=False, stop=(p == 2 and i == G - 1),
                            skip_group_check=True,
                            tile_position=(0, 32 * i))
                return po

            def epi_group(g, po):
                b, jg = divmod(g, NT // G)
                ot = opool.tile([128, F], f32, name="ot", tag="ot")
                nc.vector.tensor_scalar_add(ot[:, :], po[:, :], 0.0)
                # the final groups' DMAs are the kernel's tail: spread them
                # across four queues so their dispatches run in parallel
                # (earlier groups stay on sync where dispatch cost hides)
                if g >= NG - 2:
                    queues = [nc.sync, nc.gpsimd, nc.scalar, nc.sync]
                else:
                    queues = [nc.sync] * G
                for i in range(G):
                    j = jg * G + i
                    queues[i].dma_start(out=out[b, :, j * F:(j + 1) * F],
                                        in_=ot[32 * i:32 * i + 12, :])
                zsps.pop(2 * g, None)
                zsps.pop(2 * g + 1, None)

            # Pipeline: waves+bias for group g-1 run at the END of cycle g
            # (z(g-1) was fully evacuated during cycle g-1), so epi(g-2) is
            # DVE's first op of cycle g with its input already written, and
            # the deferred act1b(g) is ACT's first op likewise. Engine
            # queues per cycle:
            # PE  [pzA(g), mm1a(g+1), pzB(g), bias(g-1), waves(g-1),
            #      mm1b(g+1)]
            # ACT [act1b(g), zA0, zA2, act1a(g+1), zB1]
            # DVE [epi(g-2), zA1, zB0, zB2]
            wavepo = {}
            mm1_pair(0, 0)
            mm1_pair(0, 1, pool=popool)
            for g in range(NG):
                flush_act1(g)
                if g >= 2:
                    epi_group(g - 2, wavepo.pop(g - 2))
                load_group(g + 2)
                mm2_pair(g, 0)
                mm1_pair(g + 1, 0)
                mm2_pair(g, 1)
                if g >= 1:
                    bias_mm(g - 1)
                    wavepo[g - 1] = mm3_group(g - 1)
                mm1_pair(g + 1, 1, defer=True)
            bias_mm(NG - 1, pool=pypool, tag="py")
            epi_group(NG - 2, wavepo.pop(NG - 2))
            wavepo[NG - 1] = mm3_group(NG - 1)
            epi_group(NG - 1, wavepo.pop(NG - 1))
    _split_waits(nc)
    return nc


def _split_waits(nc, cap=1):
    """This container's walrus build rejects instructions carrying more than
    a small number of sync waits (matmuls: just one). Move excess waits onto
    single-wait NoOp carriers inserted before the instruction on the same
    engine — semantically identical (conjunction of waits, in-order
    sequencers)."""
    import concourse.mybir as mybir

    k = 0
    for func in nc.m.functions:
        for bb in func.blocks:
            insts = bb.instructions
            out_insts = []
            changed = False
            for inst in insts:
                si = inst.sync_info
                waits = list(si.on_wait) if si and si.on_wait else []
                if len(waits) > cap:
                    for w in waits[:-cap]:
                        d = mybir.InstNoOp(name=f"I-sw{k}", ins=[], outs=[])
                        k += 1
                        d.engine = inst.engine
                        d.sync_info = mybir.SyncInfo(on_wait=[w], on_update=[])
                        nc.register_instruction(d)
                        out_insts.append(d)
                    inst.sync_info = mybir.SyncInfo(
                        on_wait=waits[-cap:],
                        on_update=list(si.on_update) if si.on_update else [])
                    changed = True
                out_insts.append(inst)
            if changed:
                bb.instructions = out_insts


def _get_program():
    if "nc" not in _CACHE:
        _CACHE["nc"] = _build_program()
    return _CACHE["nc"]


def _prep_weights(d):
    """Fold BN into conv weights/biases; pack stationary matrices (bf16)."""
    import ml_dtypes
    f8 = np.float64
    bf = ml_dtypes.bfloat16

    def g(name):
        return np.asarray(d[name], dtype=f8)

    # shared conv + BN
    s1 = g("sh_g") / np.sqrt(g("sh_var") + EPS)                     # [64]
    W1e = g("sh_w") * s1[:, None]                                   # [64,128]
    b1e = g("sh_b") * s1 + g("sh_beta") - g("sh_mean") * s1         # [64]
    w1 = np.zeros((C_IN, 128), f8)
    w1[:, 0:HC] = W1e.T                 # col-pair slot 0 -> psum rows 0..63
    w1[:, HC:128] = W1e.T               # col-pair slot 1 -> psum rows 64..127
    b1 = np.concatenate([b1e, b1e])[:, None]                        # [128,1]

    # head first layers + BN: K=64 row-tiled stationaries; pair p's block
    # has head 2p in output cols 0..63 and head 2p+1 in cols 64..127; the
    # weights are duplicated into both sbuf row halves (T0 and T8 copies).
    s2 = g("head_g1") / np.sqrt(g("head_var1") + EPS)               # [6,64]
    W2e = g("head_w1") * s2[:, :, None]                             # [6,64,64]
    b2e = g("head_b1") * s2 + g("head_beta1") - g("head_mean1") * s2  # [6,64]
    w2 = np.zeros((C_IN, 384), f8)
    b2 = np.zeros((128, 3), f8)
    for p in range(3):
        blk = np.zeros((HC, 128), f8)
        blk[:, 0:HC] = W2e[2 * p].T
        blk[:, HC:128] = W2e[2 * p + 1].T
        w2[0:HC, 128 * p:128 * (p + 1)] = blk
        w2[HC:128, 128 * p:128 * (p + 1)] = blk
        b2[0:HC, p] = b2e[2 * p]
        b2[HC:128, p] = b2e[2 * p + 1]

    # final convs: three accumulating M=12 blocks (pair p: head 2p from z
    # rows 0..63, head 2p+1 from rows 64..127, into its channel offsets)
    names = ["hm", "reg", "height", "dim", "rot", "iou"]
    Wf = [g(n + "_w") for n in names]
    bfin = [g(n + "_b") for n in names]
    w3 = np.zeros((C_IN, 36), f8)
    b3full = np.zeros((12,), f8)
    for p in range(3):
        ha, hb = 2 * p, 2 * p + 1
        ca, cb = HEAD_OUT[ha], HEAD_OUT[hb]
        off = PAIR_OFF[p]
        w3[0:64, 12 * p + off:12 * p + off + ca] = Wf[ha].T
        w3[64:128, 12 * p + off + ca:12 * p + off + ca + cb] = Wf[hb].T
        b3full[off:off + ca] = bfin[ha]
        b3full[off + ca:off + ca + cb] = bfin[hb]
    # rank-1 bias matmul operands: row 0 of b3r x row 0 of onesk broadcasts
    # b3 (in the 4x32 col-group layout) across the whole out bank
    b3r = np.zeros((HC, 128), f8)
    for i in range(G):
        b3r[0, 32 * i:32 * i + 12] = b3full
    onesk = np.zeros((HC, F), f8)
    onesk[0, :] = 1.0

    c = np.float32
    return {"w1": w1.astype(bf), "b1": b1.astype(c), "w2": w2.astype(bf),
            "b2": b2.astype(c), "w3": w3.astype(bf), "b3r": b3r.astype(bf),
            "onesk": onesk.astype(bf)}


def _ensure_ntff_hook():
    """Install the antenv.axon_hooks NTFF-profile shim if the container's
    antenv package lacks it (profiling only; never used in grading runs)."""
    try:
        from antenv.axon_hooks import get_axon_ntff_profile_hook  # noqa: F401
        return True
    except ImportError:
        pass
    import contextlib
    import ctypes
    import sys as _sys
    import types

    so_path = "/opt/axon/libaxon_pjrt.so"
    if not os.path.exists(so_path):
        return False
    lib = ctypes.CDLL(so_path)
    if not hasattr(lib, "axon_start_nrt_profile"):
        return False
    lib.axon_start_nrt_profile.argtypes = [ctypes.POINTER(ctypes.c_int64),
                                           ctypes.c_size_t]
    lib.axon_start_nrt_profile.restype = ctypes.c_int64
    lib.axon_stop_nrt_profile.argtypes = [ctypes.c_char_p]
    lib.axon_stop_nrt_profile.restype = ctypes.c_int64

    @contextlib.contextmanager
    def _hook(output_dir, device_ids):
        import jax
        jax.devices()
        if device_ids:
            ids = (ctypes.c_int64 * len(device_ids))(*device_ids)
            rc = lib.axon_start_nrt_profile(ids, len(device_ids))
        else:
            rc = lib.axon_start_nrt_profile(None, 0)
        if rc != 0:
            raise RuntimeError(f"axon_start_nrt_profile rc={rc}")
        try:
            yield
        finally:
            n = lib.axon_stop_nrt_profile(str(output_dir).encode())
            print(f"profile: {n} file(s) written to {output_dir}",
                  file=sys.stderr)

    import antenv
    mod = types.ModuleType("antenv.axon_hooks")
    mod.get_axon_ntff_profile_hook = lambda: _hook
    mod.set_axon_ntff_profile_hook = lambda h: None
    _sys.modules["antenv.axon_hooks"] = mod
    antenv.axon_hooks = mod
    return True


def kernel(**inputs):
    global LAST_RESULTS, LAST_EXEC_NS
    import ml_dtypes
    from concourse.bass_utils import run_bass_kernel_spmd

    inputs = {k: np.asarray(v) for k, v in inputs.items()}
    weights = _prep_weights(inputs)

    ct = np.asarray(inputs["ct_feat"], dtype=np.float32)
    xs = ct.astype(ml_dtypes.bfloat16).reshape(NCORES, BC, C_IN, N)

    in_maps = [dict(weights, x=np.ascontiguousarray(xs[i]))
               for i in range(NCORES)]

    nc = _get_program()
    trace = bool(int(os.environ.get("CK_PROFILE", "0")))
    if trace:
        trace = _ensure_ntff_hook()
    res = run_bass_kernel_spmd(nc, in_maps, list(range(NCORES)), trace=trace)
    LAST_RESULTS = res
    LAST_EXEC_NS = res.exec_time_ns

    out = np.concatenate([np.asarray(res.results[i]["out"])
                          for i in range(NCORES)], axis=0)
    return out.astype(np.float32)
